# revision 3
# baseline (speedup 1.0000x reference)
"""Causal attention head on 8 TRN2 NeuronCores.

reference: out = softmax(causal((x @ wqk) @ x.T)) @ x @ wov
  x: [4096, 1024] f32, wqk/wov: [1024, 1024] f32.

Sharding: sequence-parallel on query rows with stride-8 interleave -- core m
owns global rows {m, m+8, m+16, ...} (512 rows). This balances the causal
triangle perfectly across cores AND keeps the SPMD graph identical on every
core: the causal mask depends on the core only through its input data
(a host-prepared [128, 1024] additive mask), never through the graph.

Per-core layout: 512 local rows = 4 row tiles of 128 (partition dim).
Local row tile r, local row t' -> global row 1024*r + m + 8*t'.
Row tile r attends to columns [0, 1024*(r+1)): col chunks c = 0..2r+1 of 512.
Chunks c = 2r, 2r+1 are the "diagonal" (mask halves 0/1); earlier chunks are
causally full. Each core runs an identical 20-unit S/PV schedule.

Precision: scores must be ~fp32-accurate (softmax of std~1024 logits is
argmax-sensitive; the min top-2 logit gap in this data is 0.1). Q and S
matmuls use an fp16 main pass plus two fp8-e5m2 DoubleRow correction passes
(hi*lo + lo*hi); DoubleRow runs at 2 contraction-chunks per instruction, so
the corrections cost ~0.5 of a full pass instead of 2.0. Logit error rms
~0.03 (vs 0.005 for 3-pass bf16x2 at 1.5x the cost, and 2.4 for plain
bf16). PV and OV run in plain fp16 (~3e-4 output error floor).

Schedule staggering: the PE work that depends on softmax results (P
transposes, PV) is emitted one xt-chunk later than the data dependency
requires, so VectorE/ScalarE softmax latency hides behind independent S
matmuls and the TensorE never stalls.
"""
import numpy as np
import ml_dtypes

import concourse.bass as bass
import concourse.tile as tile
from concourse import bacc, mybir
from concourse.bass_utils import run_bass_kernel_spmd
from concourse.masks import make_identity

F32 = mybir.dt.float32
F16 = mybir.dt.float16
BF16 = mybir.dt.bfloat16
E5 = mybir.dt.float8e5
DR = mybir.MatmulPerfMode.DoubleRow

N = 4096          # sequence length
D = 1024          # model dim
CORES = 8
ROWS = N // CORES  # 512 local rows per core
RT = ROWS // 128   # 4 row tiles
KC = D // 128      # 8 contraction chunks
MASK_VAL = -1e9
XV_CACHE = 16      # xv chunks cached for all row tiles
XV_GRP = 4         # xv chunks fetched per DMA beyond the cache

_F16 = np.float16
_E5 = ml_dtypes.float8_e5m2


def _split3(a):
    """f32 array -> (fp16 hi, e5m2 lo, e5m2 hi8)"""
    hi = np.asarray(a, dtype=_F16)
    lo = np.asarray(a.astype(np.float32) - hi.astype(np.float32), dtype=_E5)
    hi8 = np.asarray(hi, dtype=_E5)
    return hi, lo, hi8


def build_nc():
    nc = bacc.Bacc("TRN2", target_bir_lowering=False, debug=False,
                   num_devices=CORES)

    def inp(name, shape, dt):
        return nc.dram_tensor(name, shape, dt, kind="ExternalInput").ap()

    wqk_h = inp("wqk_h", [D, D], F16)
    wqk_l8 = inp("wqk_l8", [D, D], E5)
    wqk_h8 = inp("wqk_h8", [D, D], E5)
    xq_h = inp("xq_h", [D, ROWS], F16)
    xq_l8 = inp("xq_l8", [D, ROWS], E5)
    xq_h8 = inp("xq_h8", [D, ROWS], E5)
    xt_h = inp("xt_h", [D, N], F16)
    xt_l8 = inp("xt_l8", [D, N], E5)
    xt_h8 = inp("xt_h8", [D, N], E5)
    xv_d = inp("xv", [N, D], F16)
    wov_d = inp("wov", [D, D], F16)
    mask_d = inp("mask", [128, 1024], BF16)
    out_d = nc.dram_tensor("out", [ROWS, D], F32, kind="ExternalOutput").ap()

    # rearranged views for single-DMA chunked loads: row-block k -> free slot k
    xt_h_v = xt_h.rearrange("(k p) j -> p k j", p=128)
    xt_l8_v = xt_l8.rearrange("(k p) j -> p k j", p=128)
    xt_h8_v = xt_h8.rearrange("(k p) j -> p k j", p=128)
    xv_v = xv_d.rearrange("(g p) j -> p g j", p=128)
    wov_v = wov_d.rearrange("(k p) j -> p k j", p=128)

    with tile.TileContext(nc) as tc:
        with (
            tc.tile_pool(name="sb", bufs=1) as sb,
            tc.tile_pool(name="sb2", bufs=2) as sb2,
            tc.tile_pool(name="ps_mm", bufs=2, space="PSUM") as ps_mm,
            tc.tile_pool(name="ps_att", bufs=4, space="PSUM") as ps_att,
            tc.tile_pool(name="ps_tp", bufs=2, space="PSUM") as ps_tp,
        ):
            ident = sb.tile([128, 128], F16, tag="ident")
            make_identity(nc, ident[:])
            mask_sb = sb.tile([128, 1024], BF16, tag="mask")

            # ---- Q-side inputs (per-chunk DMAs so PE can start early) ----
            wqkh_sb = sb.tile([128, KC * D], F16, tag="wqkh_s")
            wqkl8_sb = sb.tile([128, KC * D], E5, tag="wqkl_pt")
            wqkh8_sb = sb.tile([128, KC * D], E5, tag="wqkh8_wov")
            xqh_sb = sb.tile([128, KC * ROWS], F16, tag="xqh_xvc")
            xq8_sb = sb.tile([128, 2 * KC * ROWS], E5, tag="xq8")
            QR = KC * ROWS
            for k in range(KC):
                nc.sync.dma_start(wqkh_sb[:, bass.ts(k, D)],
                                  wqk_h[bass.ts(k, 128), :])
                nc.sync.dma_start(xqh_sb[:, bass.ts(k, ROWS)],
                                  xq_h[bass.ts(k, 128), :])
            for k in range(KC):
                nc.sync.dma_start(wqkh8_sb[:, bass.ts(k, D)],
                                  wqk_h8[bass.ts(k, 128), :])
                nc.sync.dma_start(xq8_sb[:, bass.ts(k, ROWS)],
                                  xq_l8[bass.ts(k, 128), :])
                nc.sync.dma_start(wqkl8_sb[:, bass.ts(k, D)],
                                  wqk_l8[bass.ts(k, 128), :])
                nc.sync.dma_start(xq8_sb[:, QR + k * ROWS: QR + (k + 1) * ROWS],
                                  xq_h8[bass.ts(k, 128), :])

            # 3D DoubleRow views: [p, k-chunk, cols]
            wqkh8_v = wqkh8_sb[:].rearrange("p (k d) -> p k d", k=KC)
            wqkl8_v = wqkl8_sb[:].rearrange("p (k d) -> p k d", k=KC)
            xql8_v = xq8_sb[:, 0:QR].rearrange("p (k t) -> p k t", k=KC)
            xqh8_v = xq8_sb[:, QR:2 * QR].rearrange("p (k t) -> p k t", k=KC)

            # ---- Q phase: QT[d', t] chunks; fp16 main + e5 DR corrections;
            # split result into fp16 hi + e5 lo/hi8 for the S matmuls ----
            qt_h = sb.tile([128, KC * ROWS], F16, tag="qt_h")
            qt8 = sb.tile([128, 2 * KC * ROWS], E5, tag="qt8")
            for q in range(KC):
                acc = ps_mm.tile([128, ROWS], F32, tag="mm")
                for k in range(KC):
                    nc.tensor.matmul(
                        acc[:],
                        wqkh_sb[:, k * D + q * 128: k * D + (q + 1) * 128],
                        xqh_sb[:, bass.ts(k, ROWS)],
                        start=(k == 0), stop=False)
                for kk in range(KC // 2):
                    nc.tensor.matmul(
                        acc[:],
                        wqkh8_v[:, 2 * kk:2 * kk + 2,
                                q * 128:(q + 1) * 128],
                        xql8_v[:, 2 * kk:2 * kk + 2, :],
                        start=False, stop=False, perf_mode=DR)
                    nc.tensor.matmul(
                        acc[:],
                        wqkl8_v[:, 2 * kk:2 * kk + 2,
                                q * 128:(q + 1) * 128],
                        xqh8_v[:, 2 * kk:2 * kk + 2, :],
                        start=False, stop=(kk == KC // 2 - 1), perf_mode=DR)
                nc.vector.tensor_copy(qt_h[:, bass.ts(q, ROWS)], acc[:])
                nc.vector.tensor_sub(qt8[:, bass.ts(q, ROWS)], acc[:],
                                     qt_h[:, bass.ts(q, ROWS)])
                nc.vector.tensor_copy(qt8[:, QR + q * ROWS: QR + (q + 1) * ROWS],
                                      qt_h[:, bass.ts(q, ROWS)])
            qtl8_v = qt8[:, 0:QR].rearrange("p (k t) -> p k t", k=KC)
            qth8_v = qt8[:, QR:2 * QR].rearrange("p (k t) -> p k t", k=KC)

            # ---- persistent per-row-tile buffers ----
            s_off = [0, 1024, 3072, 6144]
            s_len = [(2 * r + 2) * 512 for r in range(RT)]
            s_all = sb.tile([128, 10240], F32, tag="wqkh_s")
            pt_all = sb.tile([128, 10240], F16, tag="wqkl_pt")
            xv_cache = sb.tile([128, XV_CACHE * D], F16, tag="xqh_xvc")
            nc.gpsimd.dma_start(
                xv_cache[:].rearrange("p (g j) -> p g j", g=XV_CACHE),
                xv_v[:, 0:XV_CACHE, :])

            mx_all = sb.tile([128, 8 * RT], F32, tag="mx")
            negmax = sb.tile([128, RT], F32, tag="negmax")
            lsum = sb.tile([128, RT], F32, tag="lsum")
            lpart = sb.tile([128, 2 * RT], F32, tag="lpart")
            linv = sb.tile([128, RT], F32, tag="linv")

            wov_sb = sb.tile([128, KC * D], F16, tag="wqkh8_wov")
            nc.gpsimd.dma_start(
                wov_sb[:].rearrange("p (k j) -> p k j", k=KC), wov_v[:])

            xt_cur = {}

            def dma_xt(c):
                xt16 = sb2.tile([128, KC * 512], F16, tag="xt16",
                                name=f"xt16_c{c}")
                xt8 = sb2.tile([128, 2 * KC * 512], E5, tag="xt8",
                               name=f"xt8_c{c}")
                nc.sync.dma_start(
                    xt16[:].rearrange("p (k j) -> p k j", k=KC),
                    xt_h_v[:, :, bass.ts(c, 512)])
                nc.sync.dma_start(
                    xt8[:, 0:KC * 512].rearrange("p (k j) -> p k j", k=KC),
                    xt_l8_v[:, :, bass.ts(c, 512)])
                nc.sync.dma_start(
                    xt8[:, KC * 512:].rearrange("p (k j) -> p k j", k=KC),
                    xt_h8_v[:, :, bass.ts(c, 512)])
                xt_cur["h"], xt_cur["8"] = xt16, xt8

            def s_chunk(r, c):
                acc = ps_mm.tile([128, 512], F32, tag="mm",
                                 name=f"s_r{r}c{c}")
                xt16, xt8 = xt_cur["h"], xt_cur["8"]
                xtl8_v = xt8[:, 0:KC * 512].rearrange(
                    "p (k j) -> p k j", k=KC)
                xth8_v = xt8[:, KC * 512:].rearrange(
                    "p (k j) -> p k j", k=KC)
                for k in range(KC):
                    nc.tensor.matmul(
                        acc[:],
                        qt_h[:, k * ROWS + r * 128: k * ROWS + (r + 1) * 128],
                        xt16[:, bass.ts(k, 512)],
                        start=(k == 0), stop=False)
                for kk in range(KC // 2):
                    nc.tensor.matmul(
                        acc[:],
                        qth8_v[:, 2 * kk:2 * kk + 2,
                               r * 128:(r + 1) * 128],
                        xtl8_v[:, 2 * kk:2 * kk + 2, :],
                        start=False, stop=False, perf_mode=DR)
                    nc.tensor.matmul(
                        acc[:],
                        qtl8_v[:, 2 * kk:2 * kk + 2,
                               r * 128:(r + 1) * 128],
                        xth8_v[:, 2 * kk:2 * kk + 2, :],
                        start=False, stop=(kk == KC // 2 - 1), perf_mode=DR)
                dst = s_all[:, s_off[r] + c * 512: s_off[r] + (c + 1) * 512]
                if c == 2 * r:
                    nc.vector.tensor_add(dst, acc[:], mask_sb[:, 0:512])
                elif c == 2 * r + 1:
                    nc.vector.tensor_add(dst, acc[:], mask_sb[:, 512:1024])
                else:
                    nc.scalar.copy(dst, acc[:])
                # per-chunk row max (pipelines the softmax stats)
                nc.vector.tensor_reduce(
                    out=mx_all[:, r * 8 + c: r * 8 + c + 1], in_=dst,
                    op=mybir.AluOpType.max, axis=mybir.AxisListType.X)

            p_tiles = {}
            att_tiles = {}

            def stats(r):
                """combine chunk maxes -> exp -> row sums (DVE/ACT only)"""
                nm = negmax[:, r: r + 1]
                nc.vector.tensor_reduce(
                    out=nm, in_=mx_all[:, r * 8: r * 8 + 2 * r + 2],
                    op=mybir.AluOpType.max, axis=mybir.AxisListType.X,
                    negate=True)
                p_r = sb2.tile([128, s_len[RT - 1]], F16, tag="p_r",
                               name=f"p_r{r}")
                half = (s_len[r] // 2 + 511) // 512 * 512 if s_len[r] > 2048 else s_len[r]
                pieces = [(0, half)]
                if half < s_len[r]:
                    pieces.append((half, s_len[r] - half))
                for pi, (off, ln) in enumerate(pieces):
                    nc.scalar.activation(
                        p_r[:, off: off + ln],
                        s_all[:, s_off[r] + off: s_off[r] + off + ln],
                        mybir.ActivationFunctionType.Exp,
                        bias=nm, scale=1.0,
                        accum_out=lpart[:, 2 * r + pi: 2 * r + pi + 1])
                if len(pieces) == 1:
                    nc.vector.reciprocal(linv[:, r: r + 1],
                                         lpart[:, 2 * r: 2 * r + 1])
                else:
                    nc.vector.tensor_add(lsum[:, r: r + 1],
                                         lpart[:, 2 * r: 2 * r + 1],
                                         lpart[:, 2 * r + 1: 2 * r + 2])
                    nc.vector.reciprocal(linv[:, r: r + 1], lsum[:, r: r + 1])
                p_tiles[r] = p_r

            def tpv(r):
                """P transposes + PV matmuls (PE-heavy)"""
                p_r = p_tiles[r]
                nch = s_len[r] // 128
                for g in range(nch // 4):
                    pt_ps = ps_tp.tile([128, 512], F16, tag="tp",
                                       name=f"pt_r{r}g{g}")
                    for i in range(4):
                        jc = g * 4 + i
                        nc.tensor.matmul(
                            pt_ps[:, bass.ts(i, 128)],
                            p_r[:, bass.ts(jc, 128)],
                            ident[:], is_transpose=True,
                            start=(i == 0), stop=(i == 3))
                    nc.vector.tensor_copy(
                        pt_all[:, s_off[r] + g * 512: s_off[r] + (g + 1) * 512],
                        pt_ps[:])
                njc = 8 * (r + 1)
                att_ps = [ps_att.tile([128, 512], F32, tag="att",
                                      name=f"att_r{r}h{h}")
                          for h in range(2)]
                att_tiles[r] = att_ps
                jc = 0
                while jc < njc:
                    if jc < XV_CACHE:
                        xv_t, base, span = xv_cache, 0, XV_CACHE
                    else:
                        xv_t = sb2.tile([128, XV_GRP * D], F16, tag="xv",
                                        name=f"xv_r{r}j{jc}")
                        nc.gpsimd.dma_start(
                            xv_t[:].rearrange("p (g j) -> p g j", g=XV_GRP),
                            xv_v[:, jc: jc + XV_GRP, :])
                        base, span = jc, XV_GRP
                    for j2 in range(span):
                        lhs = pt_all[:, s_off[r] + (base + j2) * 128:
                                     s_off[r] + (base + j2 + 1) * 128]
                        for h in range(2):
                            nc.tensor.matmul(
                                att_ps[h][:], lhs,
                                xv_t[:, j2 * D + h * 512: j2 * D + (h + 1) * 512],
                                start=(base + j2 == 0),
                                stop=(base + j2 == njc - 1))
                    jc = base + span

            def fin(r):
                """att normalize + transpose + OV + output DMA"""
                att_ps = att_tiles[r]
                att_sb = sb2.tile([128, D], F16, tag="att_sb", bufs=1,
                                  name=f"att_sb{r}")
                for h in range(2):
                    nc.scalar.mul(att_sb[:, bass.ts(h, 512)], att_ps[h][:],
                                  linv[:, r: r + 1])
                attT = sb2.tile([128, D], F16, tag="attT", bufs=1,
                                name=f"attT{r}")
                for g in range(2):
                    at_ps = ps_tp.tile([128, 512], F16, tag="tp",
                                       name=f"at_r{r}g{g}")
                    for i in range(4):
                        nc.tensor.matmul(at_ps[:, bass.ts(i, 128)],
                                         att_sb[:, bass.ts(g * 4 + i, 128)],
                                         ident[:], is_transpose=True,
                                         start=(i == 0), stop=(i == 3))
                    nc.vector.tensor_copy(attT[:, bass.ts(g, 512)], at_ps[:])
                out_sb = sb2.tile([128, D], F32, tag="out_sb", bufs=1,
                                  name=f"out_sb{r}")
                for h in range(2):
                    acc = ps_mm.tile([128, 512], F32, tag="mm",
                                     name=f"ov_r{r}h{h}")
                    for k in range(KC):
                        nc.tensor.matmul(
                            acc[:], attT[:, bass.ts(k, 128)],
                            wov_sb[:, k * D + h * 512: k * D + (h + 1) * 512],
                            start=(k == 0), stop=(k == KC - 1))
                    nc.scalar.copy(out_sb[:, bass.ts(h, 512)], acc[:])
                nc.gpsimd.dma_start(out_d[bass.ts(r, 128), :], out_sb[:])

            nc.gpsimd.dma_start(mask_sb[:], mask_d[:])
            # staggered schedule: S chunks stream; softmax stats right after
            # data ready; PE-dependent tpv/fin one chunk later.
            for c in range(2 * RT):
                dma_xt(c)
                for r in range(c // 2, RT):
                    s_chunk(r, c)
                if c >= 2 and c % 2 == 0:
                    tpv(c // 2 - 1)
                if c % 2 == 1:
                    stats((c - 1) // 2)
                if c >= 3 and c % 2 == 1:
                    fin(c // 2 - 1)
            tpv(RT - 1)
            fin(RT - 1)

    nc.compile()
    return nc


_NC_CACHE = {}


def _get_nc():
    if "nc" not in _NC_CACHE:
        _NC_CACHE["nc"] = build_nc()
    return _NC_CACHE["nc"]


def make_in_maps(x, wqk, wov):
    x = np.ascontiguousarray(x, dtype=np.float32)
    wqk = np.ascontiguousarray(wqk, dtype=np.float32)
    wov = np.ascontiguousarray(wov, dtype=np.float32)

    xt = np.ascontiguousarray(x.T)
    wqk_h, wqk_l8, wqk_h8 = _split3(wqk)
    xt_h, xt_l8, xt_h8 = _split3(xt)
    shared = {"xv": np.asarray(x, dtype=_F16),
              "wov": np.asarray(wov, dtype=_F16),
              "wqk_h": wqk_h, "wqk_l8": wqk_l8, "wqk_h8": wqk_h8,
              "xt_h": xt_h, "xt_l8": xt_l8, "xt_h8": xt_h8}

    in_maps = []
    t_idx = np.arange(128)
    c_idx = np.arange(1024)
    for m in range(CORES):
        xq = np.ascontiguousarray(x[m::CORES].T)
        h, l8, h8 = _split3(xq)
        mask = np.asarray(
            np.where(c_idx[None, :] <= m + 8 * t_idx[:, None],
                     0.0, MASK_VAL), dtype=ml_dtypes.bfloat16)
        im = dict(shared)
        im.update({"mask": mask, "xq_h": h, "xq_l8": l8, "xq_h8": h8})
        in_maps.append(im)
    return in_maps


def kernel(x, wqk, wov, _trace=False):
    nc = _get_nc()
    in_maps = make_in_maps(x, wqk, wov)
    res = run_bass_kernel_spmd(nc, in_maps, core_ids=list(range(CORES)),
                               trace=_trace)
    out = np.empty((N, D), dtype=np.float32)
    for m in range(CORES):
        out[m::CORES] = res.results[m]["out"]
    if _trace:
        kernel.last_results = res
    return out
